# revision 1
# baseline (speedup 1.0000x reference)
"""Graycode encoder kernel for Trainium2 (Bass/Tile), 8-core data-parallel.

Input  X: (8, 65536, 3) float32 (full).
Output:   (8, 65536, 96) int32 (full).

Per coordinate dim d (each 32 output channels):
  raw  = round(x)            (RNE, matches jnp.round)
  sign = raw > 0             -> channel 32*d
  g    = |raw| ^ (|raw| >> 1)
  bit k of g (k=0..30)       -> channel 32*d + 1 + k

Sharding: batch axis across the 8 cores (core b handles X[b]).

Per-core layout: points n = p*512 + t with p in [0,128) the SBUF partition
and t in [0,512). Processed in t-chunks (first chunk small so the output
DMA stream starts early). Bit-plane extraction uses one DVE tensor_scalar
per bit k, covering all three coordinate dims per instruction via strided
access patterns:
    in : g[p, t*3 + d]                 (dims [t, d])
    out: out[p, t*96 + 32*d + 1 + k]   (dims [t, d])

Engine split (fast path, valid when round(|x|) < 2^KB i.e. |x| < 8191.5):
  ACT   : absi = int32(|x|)        (HW converts f32->i32 with RNE)
  DVE   : g = (absi>>1)^absi; bit planes k=0..KB-2
  GpSimd: sign channel (x > 0.5); top plane k=KB-1 = (g >= 2^(KB-1));
          one-time zeroing of planes KB..30 on the persistent out buffers.
A full 31-plane variant (no range assumption) is compiled lazily if the
input exceeds the fast-path bound.
"""

import numpy as np

import concourse.tile as tile
from concourse import bacc, mybir
from concourse.bass_utils import run_bass_kernel_spmd

A = mybir.AluOpType
ACTF = mybir.ActivationFunctionType
F32, I32 = mybir.dt.float32, mybir.dt.int32

B, N, D = 8, 65536, 3
P = 128            # SBUF partitions
T = N // P         # 512 t-values per partition
CH = 96            # output channels
KB = 14            # fast path: gray bits 0..KB-1 computed, rest zero
CHUNKS = (32, 160, 160, 160)

_CACHE = {}


def _stt_int(eng, out, in0, scalar, in1, op0, op1):
    """scalar_tensor_tensor with an int32 immediate: out = (in0 op0 s) op1 in1."""
    return eng.add_instruction(
        mybir.InstTensorScalarPtr(
            name=eng.bass.get_next_instruction_name(),
            is_scalar_tensor_tensor=True,
            op0=op0,
            op1=op1,
            ins=[eng.lower_ap(in0),
                 mybir.ImmediateValue(dtype=I32, value=scalar),
                 eng.lower_ap(in1)],
            outs=[eng.lower_ap(out)],
        )
    )


def _build(full):
    key = "full" if full else "fast"
    if key in _CACHE:
        return _CACHE[key]

    maxtc = max(CHUNKS)

    nc = bacc.Bacc("TRN2", target_bir_lowering=False, debug=False, num_devices=B)
    x = nc.dram_tensor("x", [N, D], F32, kind="ExternalInput").ap()
    out = nc.dram_tensor("out", [N, CH], I32, kind="ExternalOutput").ap()

    x_r = x.rearrange("(p t) d -> p t d", p=P)        # [128, 512, 3]
    out_r = out.rearrange("(p t) j -> p t j", p=P)    # [128, 512, 96]

    with tile.TileContext(nc) as tc:
        with (
            tc.tile_pool(name="pin", bufs=2) as pin,
            tc.tile_pool(name="ptmp", bufs=2) as ptmp,
            tc.tile_pool(name="pout", bufs=1) as pout,
        ):
            # two persistent out buffers, rotated across chunks
            outbufs = []
            for nb in range(2):
                ob = pout.tile([P, maxtc * CH], I32, tag=f"outbuf{nb}")
                outbufs.append(ob)
                if not full:
                    # one-time zeroing of planes KB..30 (channels 32d+1+KB..32d+31)
                    # buf0 on DVE (gates the first out-DMA: keep it fast),
                    # buf1 on GpSimd (hidden behind chunk0/1 compute)
                    obv = ob[:].rearrange("p (t d k) -> p t d k", d=D, k=32)
                    eng = nc.vector if nb == 0 else nc.gpsimd
                    eng.memset(obv[:, :, :, 1 + KB:32], 0)

            t0 = 0
            for c, tc_sz in enumerate(CHUNKS):
                tin_full = pin.tile([P, maxtc * D], F32, tag="tin")
                tin = tin_full[:, :tc_sz * D]
                nc.sync.dma_start(
                    tin.rearrange("p (t d) -> p t d", d=D),
                    x_r[:, t0:t0 + tc_sz, :],
                )
                tin_r = tin.rearrange("p (t d) -> p t d", d=D)

                # absi = int32(round(|x|)) on ACT (RNE output conversion)
                absi_full = ptmp.tile([P, maxtc * D], I32, tag="absi")
                absi = absi_full[:, :tc_sz * D]
                nc.scalar.activation(absi, tin, ACTF.Abs)

                # g = (absi >> 1) ^ absi on DVE
                g_full = ptmp.tile([P, maxtc * D], I32, tag="g")
                g = g_full[:, :tc_sz * D]
                _stt_int(nc.vector, g, absi, 1, absi,
                         A.logical_shift_right, A.bitwise_xor)
                g_r = g.rearrange("p (t d) -> p t d", d=D)

                tout = outbufs[c % 2][:, :tc_sz * CH]
                tout_r = tout.rearrange("p (t d k) -> p t d k", d=D, k=32)

                # sign channels: round(x) > 0  <=>  x > 0.5
                # (DVE: GpSimd elementwise is ~10x slower and its SBUF port
                # traffic stalls concurrent DVE ops)
                nc.vector.tensor_scalar(tout_r[:, :, :, 0], tin_r, 0.5, None,
                                        A.is_gt)

                nbits = 31 if full else KB
                for k in range(nbits):
                    nc.vector.tensor_scalar(tout_r[:, :, :, 1 + k], g_r,
                                            k, 1, A.logical_shift_right,
                                            A.bitwise_and)

                nc.sync.dma_start(
                    out_r[:, t0:t0 + tc_sz, :],
                    tout.rearrange("p (t j) -> p t j", j=CH),
                )
                t0 += tc_sz

    nc.compile()
    _CACHE[key] = nc
    return nc


def kernel(X, **run_kwargs):
    X = np.asarray(X, dtype=np.float32)
    assert X.shape == (B, N, D), X.shape
    # fast path valid iff round(|x|) < 2^KB for every element
    full = bool(np.abs(X).max() >= (1 << KB) - 0.5)
    nc = _build(full)
    in_maps = [{"x": np.ascontiguousarray(X[b])} for b in range(B)]
    res = run_bass_kernel_spmd(nc, in_maps, core_ids=list(range(B)), **run_kwargs)
    out = np.stack([r["out"] for r in res.results], axis=0)
    if run_kwargs:
        kernel.last_result = res
    return out

